# revision 1
# baseline (speedup 1.0000x reference)
"""Trainium2 Bass kernel for nn_CP_Based (CP-decomposition interaction layer).

Math (full problem):
    t[b,f,r,u] = sum_d X[b,f,d] * K[d,r,f,u]      (B=1024, F=64, D=4, R=32, U=128)
    had[b,r,u] = prod_f t[b,f,r,u]
    out[b,u]   = sum_r had[b,r,u]

Strategy:
  * Shard batch across 8 cores (B_loc = 128 = one partition tile).
  * Feature-tripling (host-side weight repack): for a triple (f0,f1,f2),
        t3 = t[.,f0,.] * t[.,f1,.] * t[.,f2,.]
           = sum_{d3=0..63} X3[b,j,d3] * K3[d3,r,j,u]
    with X3/K3 outer products of the per-feature slices. One K=64 matmul per
    triple replaces three K=4 matmuls AND cuts the elementwise hadamard from
    63 to 21 multiplies per output element (the DVE is the bottleneck engine:
    fp32 tensor_tensor runs at 1 elem/cycle/partition @ 0.96 GHz).
    64 = 21*3 + 1: factor 21 is the lone feature 63, zero-padded to K=64.
  * PE: 2 factors run concurrently via row tiling (tile_position=(64s,0)),
    each filling a [128,1024] 2-bank psum tile per (r,u) chunk.
  * DVE: running product P[b, r*u] *= psum factor chunks (one PSUM operand
    per op is a HW limit). ScalarE initializes P for the first factor.
  * Final sum over r: strided tensor_reduce.

Host prep is pure input repacking (outer products of the small inputs,
~12M mults vs ~1.3G MACs + 270M multiplies on device).
"""

import numpy as np

B, F, D, R, U = 1024, 64, 4, 32, 128
NCORES = 8
BLOC = B // NCORES          # 128 batch rows per core
NFAC = 22                   # 21 triples + 1 padded single
NGRP = NFAC // 2            # 11 groups of 2 row-tiled factors
D3 = 64                     # contraction dim per triple (4^3)
RU = R * U                  # 4096
CHUNK = 1024                # 2 psum banks per factor-chunk
NCHUNK = RU // CHUNK        # 4

_cached = {}


def _build_nc(n_rep=1, chunk=CHUNK, unroll_reps=False):
    import concourse.bass as bass
    import concourse.mybir as mybir
    import concourse.tile as tile
    from concourse import bacc

    nch = RU // chunk
    nps = 8 // (chunk // 512)  # psum tiles to fill all 8 banks
    fp32 = mybir.dt.float32
    nc = bacc.Bacc("TRN2", target_bir_lowering=False, debug=False)

    xt_d = nc.dram_tensor("xt", [128, NGRP * BLOC], fp32, kind="ExternalInput").ap()
    kr_d = nc.dram_tensor("kr", [NGRP, 128, RU], fp32, kind="ExternalInput").ap()
    out_d = nc.dram_tensor("out", [BLOC, U], fp32, kind="ExternalOutput").ap()

    with tile.TileContext(nc) as tc:
        with (
            tc.tile_pool(name="const", bufs=1) as const_pool,
            tc.tile_pool(name="kt", bufs=3) as kpool,
            tc.tile_pool(name="prod", bufs=1) as ppool,
            tc.tile_pool(name="outp", bufs=1) as opool,
            tc.tile_pool(name="ps", bufs=nps, space="PSUM") as pspool,
        ):
            xt = const_pool.tile([128, NGRP * BLOC], fp32)
            nc.sync.dma_start(xt[:], xt_d[:])

            P = ppool.tile([128, RU], fp32)

            def body():
                for m in range(NGRP):
                    kt = kpool.tile([128, RU], fp32, tag="kt")
                    nc.sync.dma_start(kt[:], kr_d[m])
                    for c in range(nch):
                        sl = slice(c * chunk, (c + 1) * chunk)
                        ps = []
                        for s in range(2):
                            pst = pspool.tile([128, chunk], fp32, tag="ps")
                            for h in range(chunk // 512):
                                hs = slice(h * 512, (h + 1) * 512)
                                nc.tensor.matmul(
                                    pst[:, hs],
                                    xt[64 * s : 64 * s + D3, m * BLOC : (m + 1) * BLOC],
                                    kt[64 * s : 64 * s + D3, c * chunk + h * 512 : c * chunk + (h + 1) * 512],
                                    start=True,
                                    stop=True,
                                    tile_position=(64 * s, 0),
                                )
                            ps.append(pst)
                        # DVE reads at most one PSUM operand per op: chain the
                        # running product through SBUF. Init via ScalarE copy.
                        if m == 0:
                            nc.scalar.copy(P[:, sl], ps[0][:])
                        else:
                            nc.vector.tensor_mul(P[:, sl], P[:, sl], ps[0][:])
                        nc.vector.tensor_mul(P[:, sl], P[:, sl], ps[1][:])

            if n_rep == 1:
                body()
            elif unroll_reps:
                for _ in range(n_rep):
                    body()
            else:
                # benchmarking mode: repeat the (idempotent) body on-device
                with tc.For_i(0, n_rep, 1):
                    body()

            osum = opool.tile([BLOC, U], fp32)
            nc.vector.tensor_reduce(
                osum[:],
                P[:].rearrange("p (r u) -> p u r", r=R),
                axis=mybir.AxisListType.X,
                op=mybir.AluOpType.add,
            )
            nc.sync.dma_start(out_d[:], osum[:])

    nc.compile()
    return nc


def _host_prep(X, K):
    """Repack inputs: per-core X3 outer products + shared K3 outer products.

    Factor j < 21 covers features (3j, 3j+1, 3j+2) with contraction index
    d3 = 16*d0 + 4*d1 + d2; factor 21 is feature 63 (d3 = d, rest zero).
    Packed layouts match SBUF tiles directly:
      kr[m, row, r*U+u]: row = 64*s + d3 holds factor (2m+s).
      xt[row, m*BLOC+b]: same row convention.
    """
    f32 = np.float32
    NT = 21
    fa = [3 * j for j in range(NT)]

    # K3 [j, d3, r*u]
    ka = K[:, :, [3 * j for j in range(NT)], :]      # [4, 32, 21, 128] (d,r,j,u)
    kb = K[:, :, [3 * j + 1 for j in range(NT)], :]
    kc = K[:, :, [3 * j + 2 for j in range(NT)], :]
    K3 = (
        ka[:, None, None] * kb[None, :, None] * kc[None, None, :]
    )                                                # [4,4,4,32,21,128] (d0,d1,d2,r,j,u)
    K3 = K3.transpose(4, 0, 1, 2, 3, 5).reshape(NT, D3, RU)  # [j, d3, r*u]
    K3f = np.zeros((NFAC, D3, RU), dtype=f32)
    K3f[:NT] = K3
    K3f[NT, :D, :] = K[:, :, 63, :].reshape(D, RU)   # lone feature 63
    kr = np.ascontiguousarray(
        K3f.reshape(NGRP, 2, D3, RU).reshape(NGRP, 128, RU)
    )

    # X3 per core [row, m*BLOC+b]
    xts = []
    for c in range(NCORES):
        Xc = X[c * BLOC : (c + 1) * BLOC]            # [128, 64, 4] (b, f, d)
        xa = Xc[:, [3 * j for j in range(NT)], :]    # [b, j, 4]
        xb = Xc[:, [3 * j + 1 for j in range(NT)], :]
        xc = Xc[:, [3 * j + 2 for j in range(NT)], :]
        X3 = (
            xa[:, :, :, None, None] * xb[:, :, None, :, None] * xc[:, :, None, None, :]
        )                                            # [b, j, 4, 4, 4]
        X3 = X3.reshape(BLOC, NT, D3)
        X3f = np.zeros((BLOC, NFAC, D3), dtype=f32)
        X3f[:, :NT] = X3
        X3f[:, NT, :D] = Xc[:, 63, :]
        xt = X3f.transpose(1, 2, 0).reshape(NGRP, 128, BLOC)  # [m, row, b]
        xts.append(np.ascontiguousarray(xt.transpose(1, 0, 2).reshape(128, NGRP * BLOC)))
    return xts, kr


def kernel(**inputs):
    from concourse.bass_utils import run_bass_kernel_spmd

    X = np.asarray(inputs["X"], dtype=np.float32)
    K = np.asarray(inputs["kernel"], dtype=np.float32)
    assert X.shape == (B, F, D) and K.shape == (D, R, F, U)

    if "nc" not in _cached:
        _cached["nc"] = _build_nc()
    nc = _cached["nc"]

    xts, kr = _host_prep(X, K)
    in_maps = [{"xt": xts[c], "kr": kr} for c in range(NCORES)]
    res = run_bass_kernel_spmd(nc, in_maps, core_ids=list(range(NCORES)))
    return np.concatenate([res.results[c]["out"] for c in range(NCORES)], axis=0)



# revision 11
# speedup vs baseline: 1.0002x; 1.0002x over previous
"""Trainium2 Bass kernel for nn_CP_Based (CP-decomposition interaction layer).

Math (full problem):
    t[b,f,r,u] = sum_d X[b,f,d] * K[d,r,f,u]      (B=1024, F=64, D=4, R=32, U=128)
    had[b,r,u] = prod_f t[b,f,r,u]
    out[b,u]   = sum_r had[b,r,u]

Strategy (v2 — transposed layout, bf16 PE, 3-engine hadamard):
  * Feature-tripling (host-side repack): triple (f0,f1,f2) of features gives
    one K=64 contraction per factor; 22 factors total (21 triples + feature
    63 alone, zero-padded). Factor pairs share a 128-row PE pass via
    tile_position row tiling -> 11 groups.
  * Shard UNITS across 8 cores (U_loc=16): per-core kr is 1.4 MB and xt
    2.9 MB (both bf16) vs 23 MB fp32 replicated in v1 -> DMA ~13us.
  * TRANSPOSED layout: partitions = ru rows (u_loc*32+r), free dim = batch
    (1024).  4 ru-tiles of 128 rows; ops have 1024-elem free dims.
  * PE in bf16: 1 cyc/row (vs 4 for fp32) -> 45k cycles ~ 19us.
  * Hadamard split across engines (DVE tensor_tensor is 1x with a PSUM
    operand, 2x_1p for bf16 SBUF; ScalarE can copy PSUM->SBUF w/ downcast;
    GpSimd muls SBUF only at 0.42 eff):
      - DVE chain over 8 factors straight from PSUM (transit+mul in one op)
      - ScalarE copies 14 factors PSUM->SBUF bf16
      - DVE chains 8 of the copies at 2x_1p; GpSimd chains 6
      - final combine on DVE
  * r-sum for free on PE: out[u,b] = sel[ru,u].T @ P_total[ru,b] per ru-tile
    accumulated into one [16,1024] PSUM tile; host transposes per-core
    [16,1024] -> [1024,16] during gather.
"""

import numpy as np
import ml_dtypes

B, F, D, R, U = 1024, 64, 4, 32, 128
NCORES = 8
ULOC = U // NCORES          # 16 units per core
RULOC = R * ULOC            # 512 ru rows per core
NTILE = RULOC // 128        # 4 ru-tiles of 128 partitions
NT = 21                     # feature triples
NFAC = 22                   # 21 triples + 1 padded single
NGRP = NFAC // 2            # 11 groups of 2 row-tiled factors
D3 = 64                     # contraction dim per triple (4^3)
NB = B                      # full batch on the free dim
MMN = 512                   # max moving free dim per matmul

# lane assignment per factor index j = 2m+s (interleaved so the three
# engines all stay busy from the start of each ru-tile):
DVE_CHAIN = {0, 1, 6, 7, 12, 13, 18, 19}      # DVE mul straight from PSUM
DVE_TREE = {2, 3, 4, 5, 8, 9, 10, 11}         # ScalarE copy -> DVE bf16 chain
POOL_TREE = {14, 15, 16, 17, 20, 21}          # ScalarE copy -> GpSimd chain

_cached = {}


def _build_nc():
    import concourse.bass as bass
    import concourse.mybir as mybir
    import concourse.tile as tile
    from concourse import bacc

    fp32 = mybir.dt.float32
    fp32r = mybir.dt.float32r
    bf16 = mybir.dt.bfloat16
    nc = bacc.Bacc("TRN2", target_bir_lowering=False, debug=False)

    xt_d = nc.dram_tensor("xt", [128, NGRP * NB], fp32r, kind="ExternalInput").ap()
    kr_d = nc.dram_tensor("kr", [128, NGRP * RULOC], fp32r, kind="ExternalInput").ap()
    sel_d = nc.dram_tensor("sel", [128, 4 * ULOC], bf16, kind="ExternalInput").ap()
    out_d = nc.dram_tensor("out", [ULOC, NB], fp32, kind="ExternalOutput").ap()

    with tile.TileContext(nc) as tc:
        with (
            tc.tile_pool(name="const", bufs=1) as cpool,
            tc.tile_pool(name="cf", bufs=6) as cfpool,
            tc.tile_pool(name="part", bufs=2) as ppool,
            tc.tile_pool(name="ps", bufs=3, space="PSUM") as pspool,
            tc.tile_pool(name="pso", bufs=1, space="PSUM") as opool,
        ):
            xt = cpool.tile([128, NGRP * NB], fp32r)
            kr = cpool.tile([128, NGRP * RULOC], fp32r)
            sel = cpool.tile([128, 4 * ULOC], bf16)
            # split input DMAs so the first matmul only waits on a sliver
            nc.sync.dma_start(kr[:, 0 : 2 * RULOC], kr_d[:, 0 : 2 * RULOC])
            nc.sync.dma_start(xt[:, 0 : 2 * NB], xt_d[:, 0 : 2 * NB])
            nc.sync.dma_start(sel[:], sel_d[:])
            nc.sync.dma_start(kr[:, 2 * RULOC :], kr_d[:, 2 * RULOC :])
            nc.sync.dma_start(xt[:, 2 * NB :], xt_d[:, 2 * NB :])

            out_ps = opool.tile([ULOC, NB], fp32)
            pending = []  # deferred r-sum matmuls: (rt, P_tot tile)

            def emit_rsum(rt, ptot):
                # accumulate all 4 ru-tiles into one [16, NB] psum tile; the
                # per-tile sel slice is nonzero only in columns 4rt..4rt+3
                for h in range(NB // MMN):
                    hs = slice(h * MMN, (h + 1) * MMN)
                    nc.tensor.matmul(
                        out_ps[:, hs],
                        sel[:, ULOC * rt : ULOC * rt + ULOC],
                        ptot[:, hs],
                        start=(rt == 0),
                        stop=(rt == NTILE - 1),
                        skip_group_check=True,
                    )

            for rt in range(NTILE):
                P_dve = ppool.tile([128, NB], fp32, tag="pdve")
                P_act = ppool.tile([128, NB], bf16, tag="pact")
                P_pool = ppool.tile([128, NB], bf16, tag="ppool")
                X1 = ppool.tile([128, NB], bf16, tag="x1")
                P_tot = ppool.tile([128, NB], bf16, tag="ptot")
                n_act = 0   # copied factors so far, per tree
                n_pool = 0

                for m in range(NGRP):
                    fac = []
                    for s in range(2):
                        pst = pspool.tile([128, NB], fp32, tag="ps")
                        for h in range(NB // MMN):
                            hs = slice(h * MMN, (h + 1) * MMN)
                            nc.tensor.matmul(
                                pst[:, hs],
                                kr[
                                    64 * s : 64 * s + D3,
                                    m * RULOC + 128 * rt : m * RULOC + 128 * rt + 128,
                                ],
                                xt[64 * s : 64 * s + D3, m * NB + h * MMN : m * NB + (h + 1) * MMN],
                                start=True,
                                stop=True,
                                tile_position=(64 * s, 0),
                            )
                        fac.append(pst)
                    # drain the previous ru-tile's r-sum once this tile's PE
                    # stream is 3 groups in (so PE doesn't stall on it)
                    if m == 2 and pending:
                        emit_rsum(*pending.pop())

                    for s in range(2):
                        j = 2 * m + s
                        Fj = fac[s]
                        if j in DVE_CHAIN:
                            if j == 0:
                                nc.vector.tensor_copy(P_dve[:], Fj[:])
                            else:
                                nc.vector.tensor_mul(P_dve[:], P_dve[:], Fj[:])
                        else:
                            cf = cfpool.tile([128, NB], bf16, tag="cf")
                            nc.scalar.copy(cf[:], Fj[:])
                            if j in DVE_TREE:
                                if n_act == 0:
                                    first_act = cf
                                elif n_act == 1:
                                    nc.vector.tensor_mul(P_act[:], first_act[:], cf[:])
                                else:
                                    nc.vector.tensor_mul(P_act[:], P_act[:], cf[:])
                                n_act += 1
                            else:
                                if n_pool == 0:
                                    first_pool = cf
                                elif n_pool == 1:
                                    nc.gpsimd.tensor_mul(P_pool[:], first_pool[:], cf[:])
                                else:
                                    nc.gpsimd.tensor_mul(P_pool[:], P_pool[:], cf[:])
                                n_pool += 1

                nc.gpsimd.tensor_mul(X1[:], P_act[:], P_pool[:])
                nc.vector.tensor_mul(P_tot[:], P_dve[:], X1[:])
                pending.append((rt, P_tot))

            while pending:
                emit_rsum(*pending.pop())
            osb = cpool.tile([ULOC, NB], fp32)
            nc.scalar.copy(osb[:], out_ps[:])
            nc.sync.dma_start(out_d[:], osb[:])

    nc.compile()
    return nc


def _host_prep(X, K):
    """Repack inputs (all bf16):
      xt[row, m*NB + b]        : X3 outer products; row = 64*s + d3 holds
                                 factor j=2m+s; d3 = 16*d0+4*d1+d2.
      kr_c[row, m*RULOC + u_loc*32 + r] : K3 outer products, u-sliced per core.
      sel[k, t] = 1 if k//32 == t      : r-sum selection matrix.
    """
    f32 = np.float32
    bf16 = ml_dtypes.bfloat16

    xa = X[:, [3 * j for j in range(NT)], :]         # [B, 21, 4]
    xb = X[:, [3 * j + 1 for j in range(NT)], :]
    xc = X[:, [3 * j + 2 for j in range(NT)], :]
    X3 = (
        xa[:, :, :, None, None] * xb[:, :, None, :, None] * xc[:, :, None, None, :]
    ).reshape(B, NT, D3)                             # [B, 21, 64]
    X3f = np.zeros((B, NFAC, D3), dtype=f32)
    X3f[:, :NT] = X3
    X3f[:, NT, :D] = X[:, 63, :]
    # -> xt[row, m*NB+b]: [NFAC, D3, B] -> [NGRP, 2, D3, B] -> [128, NGRP*B]
    xt = (
        X3f.transpose(1, 2, 0)
        .reshape(NGRP, 2 * D3, B)
        .transpose(1, 0, 2)
        .reshape(2 * D3, NGRP * B)
    )
    xt = np.ascontiguousarray(xt)  # float32r = fp32 bits, reduced-precision mode

    ka = K[:, :, [3 * j for j in range(NT)], :]      # [4, 32, 21, 128] (d,r,j,u)
    kb = K[:, :, [3 * j + 1 for j in range(NT)], :]
    kc = K[:, :, [3 * j + 2 for j in range(NT)], :]
    K3 = (
        ka[:, None, None] * kb[None, :, None] * kc[None, None, :]
    )                                                # [4,4,4,32,21,128] (d0,d1,d2,r,j,u)
    K3 = K3.transpose(4, 0, 1, 2, 3, 5).reshape(NT, D3, R, U)  # [j, d3, r, u]
    K3f = np.zeros((NFAC, D3, R, U), dtype=f32)
    K3f[:NT] = K3
    K3f[NT, :D] = K[:, :, 63, :]                     # lone feature 63
    krs = []
    for c in range(NCORES):
        Kc = K3f[:, :, :, c * ULOC : (c + 1) * ULOC]   # [NFAC, D3, R, ULOC]
        # cols ordered u_loc*32 + r  -> [NFAC, D3, ULOC, R]
        Kc = Kc.transpose(0, 1, 3, 2).reshape(NFAC, D3, RULOC)
        kr = (
            Kc.reshape(NGRP, 2, D3, RULOC)
            .transpose(1, 2, 0, 3)
            .reshape(2 * D3, NGRP * RULOC)
        )
        krs.append(np.ascontiguousarray(kr))

    selmat = np.zeros((128, 4 * ULOC), dtype=bf16)
    for rt in range(NTILE):
        for k in range(128):
            selmat[k, ULOC * rt + 4 * rt + k // 32] = 1
    return xt, krs, selmat


def kernel(**inputs):
    from concourse.bass_utils import run_bass_kernel_spmd

    X = np.asarray(inputs["X"], dtype=np.float32)
    K = np.asarray(inputs["kernel"], dtype=np.float32)
    assert X.shape == (B, F, D) and K.shape == (D, R, F, U)

    if "nc" not in _cached:
        _cached["nc"] = _build_nc()
    nc = _cached["nc"]

    xt, krs, selmat = _host_prep(X, K)
    in_maps = [{"xt": xt, "kr": krs[c], "sel": selmat} for c in range(NCORES)]
    res = run_bass_kernel_spmd(nc, in_maps, core_ids=list(range(NCORES)))
    out = np.empty((B, U), dtype=np.float32)
    for c in range(NCORES):
        out[:, c * ULOC : (c + 1) * ULOC] = np.asarray(
            res.results[c]["out"], dtype=np.float32
        ).T
    return out


# revision 15
# speedup vs baseline: 1.0766x; 1.0764x over previous
"""Trainium2 Bass kernel for nn_CP_Based (CP-decomposition interaction layer).

Math (full problem):
    t[b,f,r,u] = sum_d X[b,f,d] * K[d,r,f,u]      (B=1024, F=64, D=4, R=32, U=128)
    had[b,r,u] = prod_f t[b,f,r,u]
    out[b,u]   = sum_r had[b,r,u]

Strategy (v2 — transposed layout, bf16 PE, 3-engine hadamard):
  * Feature-tripling (host-side repack): triple (f0,f1,f2) of features gives
    one K=64 contraction per factor; 22 factors total (21 triples + feature
    63 alone, zero-padded). Factor pairs share a 128-row PE pass via
    tile_position row tiling -> 11 groups.
  * Shard UNITS across 8 cores (U_loc=16): per-core kr is 1.4 MB and xt
    2.9 MB (both bf16) vs 23 MB fp32 replicated in v1 -> DMA ~13us.
  * TRANSPOSED layout: partitions = ru rows (u_loc*32+r), free dim = batch
    (1024).  4 ru-tiles of 128 rows; ops have 1024-elem free dims.
  * PE in bf16: 1 cyc/row (vs 4 for fp32) -> 45k cycles ~ 19us.
  * Hadamard split across engines (DVE tensor_tensor is 1x with a PSUM
    operand, 2x_1p for bf16 SBUF; ScalarE can copy PSUM->SBUF w/ downcast;
    GpSimd muls SBUF only at 0.42 eff):
      - DVE chain over 8 factors straight from PSUM (transit+mul in one op)
      - ScalarE copies 14 factors PSUM->SBUF bf16
      - DVE chains 8 of the copies at 2x_1p; GpSimd chains 6
      - final combine on DVE
  * r-sum for free on PE: out[u,b] = sel[ru,u].T @ P_total[ru,b] per ru-tile
    accumulated into one [16,1024] PSUM tile; host transposes per-core
    [16,1024] -> [1024,16] during gather.
"""

import numpy as np

B, F, D, R, U = 1024, 64, 4, 32, 128
NCORES = 8
ULOC = U // NCORES          # 16 units per core
RULOC = R * ULOC            # 512 ru rows per core
NTILE = RULOC // 128        # 4 ru-tiles of 128 partitions
NT = 21                     # feature triples
NFAC = 22                   # 21 triples + 1 padded single
NGRP = NFAC // 2            # 11 groups of 2 row-tiled factors
D3 = 64                     # contraction dim per triple (4^3)
NB = B                      # full batch on the free dim
MMN = 512                   # max moving free dim per matmul

# lane assignment per factor index j = 2m+s (interleaved so the three
# engines all stay busy from the start of each ru-tile):
DVE_CHAIN = {0, 1, 6, 7, 12, 13, 18, 19}      # DVE mul straight from PSUM
DVE_TREE = {2, 3, 4, 5, 8, 9, 10, 11}         # ScalarE copy -> DVE bf16 chain
POOL_TREE = {14, 15, 16, 17, 20, 21}          # ScalarE copy -> GpSimd chain

_cached = {}


def _build_nc():
    import concourse.bass as bass
    import concourse.mybir as mybir
    import concourse.tile as tile
    from concourse import bacc

    fp32 = mybir.dt.float32
    fp16 = mybir.dt.float16
    nc = bacc.Bacc("TRN2", target_bir_lowering=False, debug=False)

    xt_d = nc.dram_tensor("xt", [128, NGRP * NB], fp16, kind="ExternalInput").ap()
    kr_d = nc.dram_tensor("kr", [128, NGRP * RULOC], fp16, kind="ExternalInput").ap()
    sel_d = nc.dram_tensor("sel", [128, 4 * ULOC], fp16, kind="ExternalInput").ap()
    out_d = nc.dram_tensor("out", [ULOC, NB], fp32, kind="ExternalOutput").ap()

    with tile.TileContext(nc) as tc:
        with (
            tc.tile_pool(name="const", bufs=1) as cpool,
            tc.tile_pool(name="cf", bufs=6) as cfpool,
            tc.tile_pool(name="part", bufs=2) as ppool,
            tc.tile_pool(name="ps", bufs=3, space="PSUM") as pspool,
            tc.tile_pool(name="pso", bufs=1, space="PSUM") as opool,
        ):
            xt = cpool.tile([128, NGRP * NB], fp16)
            kr = cpool.tile([128, NGRP * RULOC], fp16)
            sel = cpool.tile([128, 4 * ULOC], fp16)
            # split input DMAs so the first matmul only waits on a sliver
            nc.sync.dma_start(kr[:, 0 : 2 * RULOC], kr_d[:, 0 : 2 * RULOC])
            nc.sync.dma_start(xt[:, 0 : 2 * NB], xt_d[:, 0 : 2 * NB])
            nc.sync.dma_start(sel[:], sel_d[:])
            nc.sync.dma_start(kr[:, 2 * RULOC :], kr_d[:, 2 * RULOC :])
            nc.sync.dma_start(xt[:, 2 * NB :], xt_d[:, 2 * NB :])

            out_ps = opool.tile([ULOC, NB], fp32)
            pending = []  # deferred r-sum matmuls: (rt, P_tot tile)

            def emit_rsum(rt, ptot):
                # accumulate all 4 ru-tiles into one [16, NB] psum tile; the
                # per-tile sel slice is nonzero only in columns 4rt..4rt+3
                for h in range(NB // MMN):
                    hs = slice(h * MMN, (h + 1) * MMN)
                    nc.tensor.matmul(
                        out_ps[:, hs],
                        sel[:, ULOC * rt : ULOC * rt + ULOC],
                        ptot[:, hs],
                        start=(rt == 0),
                        stop=(rt == NTILE - 1),
                        skip_group_check=True,
                    )

            for rt in range(NTILE):
                P_dve = ppool.tile([128, NB], fp32, tag="pdve")
                P_act = ppool.tile([128, NB], fp16, tag="pact")
                P_pool = ppool.tile([128, NB], fp16, tag="ppool")
                X1 = ppool.tile([128, NB], fp16, tag="x1")
                P_tot = ppool.tile([128, NB], fp16, tag="ptot")
                n_act = 0   # copied factors so far, per tree
                n_pool = 0

                for m in range(NGRP):
                    fac = []
                    for s in range(2):
                        pst = pspool.tile([128, NB], fp32, tag="ps")
                        for h in range(NB // MMN):
                            hs = slice(h * MMN, (h + 1) * MMN)
                            nc.tensor.matmul(
                                pst[:, hs],
                                kr[
                                    64 * s : 64 * s + D3,
                                    m * RULOC + 128 * rt : m * RULOC + 128 * rt + 128,
                                ],
                                xt[64 * s : 64 * s + D3, m * NB + h * MMN : m * NB + (h + 1) * MMN],
                                start=True,
                                stop=True,
                                tile_position=(64 * s, 0),
                            )
                        fac.append(pst)
                    # drain the previous ru-tile's r-sum once this tile's PE
                    # stream is 3 groups in (so PE doesn't stall on it)
                    if m == 2 and pending:
                        emit_rsum(*pending.pop())

                    for s in range(2):
                        j = 2 * m + s
                        Fj = fac[s]
                        if j in DVE_CHAIN:
                            if j == 0:
                                nc.vector.tensor_copy(P_dve[:], Fj[:])
                            else:
                                nc.vector.tensor_mul(P_dve[:], P_dve[:], Fj[:])
                        else:
                            cf = cfpool.tile([128, NB], fp16, tag="cf")
                            nc.scalar.copy(cf[:], Fj[:])
                            if j in DVE_TREE:
                                if n_act == 0:
                                    first_act = cf
                                elif n_act == 1:
                                    nc.vector.tensor_mul(P_act[:], first_act[:], cf[:])
                                else:
                                    nc.vector.tensor_mul(P_act[:], P_act[:], cf[:])
                                n_act += 1
                            else:
                                if n_pool == 0:
                                    first_pool = cf
                                elif n_pool == 1:
                                    nc.gpsimd.tensor_mul(P_pool[:], first_pool[:], cf[:])
                                else:
                                    nc.gpsimd.tensor_mul(P_pool[:], P_pool[:], cf[:])
                                n_pool += 1

                nc.gpsimd.tensor_mul(X1[:], P_act[:], P_pool[:])
                nc.vector.tensor_mul(P_tot[:], P_dve[:], X1[:])
                pending.append((rt, P_tot))

            while pending:
                emit_rsum(*pending.pop())
            osb = cpool.tile([ULOC, NB], fp32)
            nc.scalar.copy(osb[:], out_ps[:])
            nc.sync.dma_start(out_d[:], osb[:])

    nc.compile()
    return nc


def _host_prep(X, K):
    """Repack inputs (all fp16):
      xt[row, m*NB + b]        : X3 outer products; row = 64*s + d3 holds
                                 factor j=2m+s; d3 = 16*d0+4*d1+d2.
      kr_c[row, m*RULOC + u_loc*32 + r] : K3 outer products, u-sliced per core.
      sel[k, t] = 1 if k//32 == t      : r-sum selection matrix.
    """
    f32 = np.float32

    xa = X[:, [3 * j for j in range(NT)], :]         # [B, 21, 4]
    xb = X[:, [3 * j + 1 for j in range(NT)], :]
    xc = X[:, [3 * j + 2 for j in range(NT)], :]
    X3 = (
        xa[:, :, :, None, None] * xb[:, :, None, :, None] * xc[:, :, None, None, :]
    ).reshape(B, NT, D3)                             # [B, 21, 64]
    X3f = np.zeros((B, NFAC, D3), dtype=f32)
    X3f[:, :NT] = X3
    X3f[:, NT, :D] = X[:, 63, :]
    # -> xt[row, m*NB+b]: [NFAC, D3, B] -> [NGRP, 2, D3, B] -> [128, NGRP*B]
    xt = (
        X3f.transpose(1, 2, 0)
        .reshape(NGRP, 2 * D3, B)
        .transpose(1, 0, 2)
        .reshape(2 * D3, NGRP * B)
    )
    xt = np.ascontiguousarray(xt).astype(np.float16)

    ka = K[:, :, [3 * j for j in range(NT)], :]      # [4, 32, 21, 128] (d,r,j,u)
    kb = K[:, :, [3 * j + 1 for j in range(NT)], :]
    kc = K[:, :, [3 * j + 2 for j in range(NT)], :]
    K3 = (
        ka[:, None, None] * kb[None, :, None] * kc[None, None, :]
    )                                                # [4,4,4,32,21,128] (d0,d1,d2,r,j,u)
    K3 = K3.transpose(4, 0, 1, 2, 3, 5).reshape(NT, D3, R, U)  # [j, d3, r, u]
    K3f = np.zeros((NFAC, D3, R, U), dtype=f32)
    K3f[:NT] = K3
    K3f[NT, :D] = K[:, :, 63, :]                     # lone feature 63
    krs = []
    for c in range(NCORES):
        Kc = K3f[:, :, :, c * ULOC : (c + 1) * ULOC]   # [NFAC, D3, R, ULOC]
        # cols ordered u_loc*32 + r  -> [NFAC, D3, ULOC, R]
        Kc = Kc.transpose(0, 1, 3, 2).reshape(NFAC, D3, RULOC)
        kr = (
            Kc.reshape(NGRP, 2, D3, RULOC)
            .transpose(1, 2, 0, 3)
            .reshape(2 * D3, NGRP * RULOC)
        )
        krs.append(np.ascontiguousarray(kr).astype(np.float16))

    selmat = np.zeros((128, 4 * ULOC), dtype=np.float16)
    for rt in range(NTILE):
        for k in range(128):
            selmat[k, ULOC * rt + 4 * rt + k // 32] = 1
    return xt, krs, selmat


def kernel(**inputs):
    from concourse.bass_utils import run_bass_kernel_spmd

    X = np.asarray(inputs["X"], dtype=np.float32)
    K = np.asarray(inputs["kernel"], dtype=np.float32)
    assert X.shape == (B, F, D) and K.shape == (D, R, F, U)

    if "nc" not in _cached:
        _cached["nc"] = _build_nc()
    nc = _cached["nc"]

    xt, krs, selmat = _host_prep(X, K)
    in_maps = [{"xt": xt, "kr": krs[c], "sel": selmat} for c in range(NCORES)]
    res = run_bass_kernel_spmd(nc, in_maps, core_ids=list(range(NCORES)))
    out = np.empty((B, U), dtype=np.float32)
    for c in range(NCORES):
        out[:, c * ULOC : (c + 1) * ULOC] = np.asarray(
            res.results[c]["out"], dtype=np.float32
        ).T
    return out


# revision 17
# speedup vs baseline: 1.1456x; 1.0641x over previous
"""Trainium2 Bass kernel for nn_CP_Based (CP-decomposition interaction layer).

Math (full problem):
    t[b,f,r,u] = sum_d X[b,f,d] * K[d,r,f,u]      (B=1024, F=64, D=4, R=32, U=128)
    had[b,r,u] = prod_f t[b,f,r,u]
    out[b,u]   = sum_r had[b,r,u]

Strategy (v2 — transposed layout, bf16 PE, 3-engine hadamard):
  * Feature-tripling (host-side repack): triple (f0,f1,f2) of features gives
    one K=64 contraction per factor; 22 factors total (21 triples + feature
    63 alone, zero-padded). Factor pairs share a 128-row PE pass via
    tile_position row tiling -> 11 groups.
  * Shard UNITS across 8 cores (U_loc=16): per-core kr is 1.4 MB and xt
    2.9 MB (both bf16) vs 23 MB fp32 replicated in v1 -> DMA ~13us.
  * TRANSPOSED layout: partitions = ru rows (u_loc*32+r), free dim = batch
    (1024).  4 ru-tiles of 128 rows; ops have 1024-elem free dims.
  * PE in bf16: 1 cyc/row (vs 4 for fp32) -> 45k cycles ~ 19us.
  * Hadamard split across engines (DVE tensor_tensor is 1x with a PSUM
    operand, 2x_1p for bf16 SBUF; ScalarE can copy PSUM->SBUF w/ downcast;
    GpSimd muls SBUF only at 0.42 eff):
      - DVE chain over 8 factors straight from PSUM (transit+mul in one op)
      - ScalarE copies 14 factors PSUM->SBUF bf16
      - DVE chains 8 of the copies at 2x_1p; GpSimd chains 6
      - final combine on DVE
  * r-sum for free on PE: out[u,b] = sel[ru,u].T @ P_total[ru,b] per ru-tile
    accumulated into one [16,1024] PSUM tile; host transposes per-core
    [16,1024] -> [1024,16] during gather.
"""

import numpy as np

B, F, D, R, U = 1024, 64, 4, 32, 128
NCORES = 8
ULOC = U // NCORES          # 16 units per core
RULOC = R * ULOC            # 512 ru rows per core
NTILE = RULOC // 128        # 4 ru-tiles of 128 partitions
NT = 21                     # feature triples
NFAC = 22                   # 21 triples + 1 padded single
NGRP = NFAC // 2            # 11 groups of 2 row-tiled factors
D3 = 64                     # contraction dim per triple (4^3)
NB = B                      # full batch on the free dim
MMN = 512                   # max moving free dim per matmul

# lane assignment per factor index j = 2m+s (interleaved so the three
# engines all stay busy from the start of each ru-tile):
DVE_CHAIN = {0, 1, 6, 7, 12, 13, 18, 19}      # DVE mul straight from PSUM
DVE_TREE = {2, 3, 4, 5, 8, 9, 10, 11}         # ScalarE copy -> DVE bf16 chain
POOL_TREE = {14, 15, 16, 17, 20, 21}          # ScalarE copy -> GpSimd chain

_cached = {}


def _build_nc():
    import concourse.bass as bass
    import concourse.mybir as mybir
    import concourse.tile as tile
    from concourse import bacc

    fp32 = mybir.dt.float32
    fp16 = mybir.dt.float16
    nc = bacc.Bacc("TRN2", target_bir_lowering=False, debug=False)

    xt_d = nc.dram_tensor("xt", [128, NGRP * NB], fp16, kind="ExternalInput").ap()
    kr_d = nc.dram_tensor("kr", [128, NGRP * RULOC], fp16, kind="ExternalInput").ap()
    sel_d = nc.dram_tensor("sel", [128, 4 * ULOC], fp16, kind="ExternalInput").ap()
    out_d = nc.dram_tensor("out", [ULOC, NB], fp32, kind="ExternalOutput").ap()

    with tile.TileContext(nc) as tc:
        with (
            tc.tile_pool(name="const", bufs=1) as cpool,
            tc.tile_pool(name="cf", bufs=6) as cfpool,
            tc.tile_pool(name="part", bufs=2) as ppool,
            tc.tile_pool(name="ps", bufs=3, space="PSUM") as pspool,
            tc.tile_pool(name="pso", bufs=1, space="PSUM") as opool,
        ):
            xt = cpool.tile([128, NGRP * NB], fp16)
            kr = cpool.tile([128, NGRP * RULOC], fp16)
            sel = cpool.tile([128, 4 * ULOC], fp16)
            # split input DMAs per group-pair so group m's matmuls only wait
            # on their own slice (a monolithic transfer stalls the PE ~7us)
            nc.sync.dma_start(kr[:, 0 : 2 * RULOC], kr_d[:, 0 : 2 * RULOC])
            nc.sync.dma_start(xt[:, 0 : 2 * NB], xt_d[:, 0 : 2 * NB])
            nc.sync.dma_start(sel[:], sel_d[:])
            for g in range(2, NGRP, 2):
                g2 = min(g + 2, NGRP)
                nc.sync.dma_start(
                    kr[:, g * RULOC : g2 * RULOC], kr_d[:, g * RULOC : g2 * RULOC]
                )
                nc.sync.dma_start(
                    xt[:, g * NB : g2 * NB], xt_d[:, g * NB : g2 * NB]
                )

            out_ps = opool.tile([ULOC, NB], fp32)
            pending = []  # deferred r-sum matmuls: (rt, P_tot tile)

            def emit_rsum(rt, ptot):
                # accumulate all 4 ru-tiles into one [16, NB] psum tile; the
                # per-tile sel slice is nonzero only in columns 4rt..4rt+3
                for h in range(NB // MMN):
                    hs = slice(h * MMN, (h + 1) * MMN)
                    nc.tensor.matmul(
                        out_ps[:, hs],
                        sel[:, ULOC * rt : ULOC * rt + ULOC],
                        ptot[:, hs],
                        start=(rt == 0),
                        stop=(rt == NTILE - 1),
                        skip_group_check=True,
                    )

            for rt in range(NTILE):
                P_dve = ppool.tile([128, NB], fp32, tag="pdve")
                P_act = ppool.tile([128, NB], fp16, tag="pact")
                P_pool = ppool.tile([128, NB], fp16, tag="ppool")
                X1 = ppool.tile([128, NB], fp16, tag="x1")
                P_tot = ppool.tile([128, NB], fp16, tag="ptot")
                n_act = 0   # copied factors so far, per tree
                n_pool = 0

                for m in range(NGRP):
                    fac = []
                    for s in range(2):
                        pst = pspool.tile([128, NB], fp32, tag="ps")
                        for h in range(NB // MMN):
                            hs = slice(h * MMN, (h + 1) * MMN)
                            nc.tensor.matmul(
                                pst[:, hs],
                                kr[
                                    64 * s : 64 * s + D3,
                                    m * RULOC + 128 * rt : m * RULOC + 128 * rt + 128,
                                ],
                                xt[64 * s : 64 * s + D3, m * NB + h * MMN : m * NB + (h + 1) * MMN],
                                start=True,
                                stop=True,
                                tile_position=(64 * s, 0),
                            )
                        fac.append(pst)
                    # drain the previous ru-tile's r-sum once this tile's PE
                    # stream is 9 groups in: the previous tile's consumer
                    # chain (which P_tot waits on) lags the PE by ~8us
                    if m == 8 and pending:
                        emit_rsum(*pending.pop())

                    for s in range(2):
                        j = 2 * m + s
                        Fj = fac[s]
                        if j in DVE_CHAIN:
                            if j == 0:
                                nc.vector.tensor_copy(P_dve[:], Fj[:])
                            else:
                                nc.vector.tensor_mul(P_dve[:], P_dve[:], Fj[:])
                        else:
                            cf = cfpool.tile([128, NB], fp16, tag="cf")
                            nc.scalar.copy(cf[:], Fj[:])
                            if j in DVE_TREE:
                                if n_act == 0:
                                    first_act = cf
                                elif n_act == 1:
                                    nc.vector.tensor_mul(P_act[:], first_act[:], cf[:])
                                else:
                                    nc.vector.tensor_mul(P_act[:], P_act[:], cf[:])
                                n_act += 1
                            else:
                                if n_pool == 0:
                                    first_pool = cf
                                elif n_pool == 1:
                                    nc.gpsimd.tensor_mul(P_pool[:], first_pool[:], cf[:])
                                else:
                                    nc.gpsimd.tensor_mul(P_pool[:], P_pool[:], cf[:])
                                n_pool += 1

                nc.gpsimd.tensor_mul(X1[:], P_act[:], P_pool[:])
                nc.vector.tensor_mul(P_tot[:], P_dve[:], X1[:])
                pending.append((rt, P_tot))

            while pending:
                emit_rsum(*pending.pop())
            osb = cpool.tile([ULOC, NB], fp32)
            nc.scalar.copy(osb[:], out_ps[:])
            nc.sync.dma_start(out_d[:], osb[:])

    nc.compile()
    return nc


def _host_prep(X, K):
    """Repack inputs (all fp16):
      xt[row, m*NB + b]        : X3 outer products; row = 64*s + d3 holds
                                 factor j=2m+s; d3 = 16*d0+4*d1+d2.
      kr_c[row, m*RULOC + u_loc*32 + r] : K3 outer products, u-sliced per core.
      sel[k, t] = 1 if k//32 == t      : r-sum selection matrix.
    """
    f32 = np.float32

    xa = X[:, [3 * j for j in range(NT)], :]         # [B, 21, 4]
    xb = X[:, [3 * j + 1 for j in range(NT)], :]
    xc = X[:, [3 * j + 2 for j in range(NT)], :]
    X3 = (
        xa[:, :, :, None, None] * xb[:, :, None, :, None] * xc[:, :, None, None, :]
    ).reshape(B, NT, D3)                             # [B, 21, 64]
    X3f = np.zeros((B, NFAC, D3), dtype=f32)
    X3f[:, :NT] = X3
    X3f[:, NT, :D] = X[:, 63, :]
    # -> xt[row, m*NB+b]: [NFAC, D3, B] -> [NGRP, 2, D3, B] -> [128, NGRP*B]
    xt = (
        X3f.transpose(1, 2, 0)
        .reshape(NGRP, 2 * D3, B)
        .transpose(1, 0, 2)
        .reshape(2 * D3, NGRP * B)
    )
    xt = np.ascontiguousarray(xt).astype(np.float16)

    ka = K[:, :, [3 * j for j in range(NT)], :]      # [4, 32, 21, 128] (d,r,j,u)
    kb = K[:, :, [3 * j + 1 for j in range(NT)], :]
    kc = K[:, :, [3 * j + 2 for j in range(NT)], :]
    K3 = (
        ka[:, None, None] * kb[None, :, None] * kc[None, None, :]
    )                                                # [4,4,4,32,21,128] (d0,d1,d2,r,j,u)
    K3 = K3.transpose(4, 0, 1, 2, 3, 5).reshape(NT, D3, R, U)  # [j, d3, r, u]
    K3f = np.zeros((NFAC, D3, R, U), dtype=f32)
    K3f[:NT] = K3
    K3f[NT, :D] = K[:, :, 63, :]                     # lone feature 63
    krs = []
    for c in range(NCORES):
        Kc = K3f[:, :, :, c * ULOC : (c + 1) * ULOC]   # [NFAC, D3, R, ULOC]
        # cols ordered u_loc*32 + r  -> [NFAC, D3, ULOC, R]
        Kc = Kc.transpose(0, 1, 3, 2).reshape(NFAC, D3, RULOC)
        kr = (
            Kc.reshape(NGRP, 2, D3, RULOC)
            .transpose(1, 2, 0, 3)
            .reshape(2 * D3, NGRP * RULOC)
        )
        krs.append(np.ascontiguousarray(kr).astype(np.float16))

    selmat = np.zeros((128, 4 * ULOC), dtype=np.float16)
    for rt in range(NTILE):
        for k in range(128):
            selmat[k, ULOC * rt + 4 * rt + k // 32] = 1
    return xt, krs, selmat


def kernel(**inputs):
    from concourse.bass_utils import run_bass_kernel_spmd

    X = np.asarray(inputs["X"], dtype=np.float32)
    K = np.asarray(inputs["kernel"], dtype=np.float32)
    assert X.shape == (B, F, D) and K.shape == (D, R, F, U)

    if "nc" not in _cached:
        _cached["nc"] = _build_nc()
    nc = _cached["nc"]

    xt, krs, selmat = _host_prep(X, K)
    in_maps = [{"xt": xt, "kr": krs[c], "sel": selmat} for c in range(NCORES)]
    res = run_bass_kernel_spmd(nc, in_maps, core_ids=list(range(NCORES)))
    out = np.empty((B, U), dtype=np.float32)
    for c in range(NCORES):
        out[:, c * ULOC : (c + 1) * ULOC] = np.asarray(
            res.results[c]["out"], dtype=np.float32
        ).T
    return out
